# revision 1
# baseline (speedup 1.0000x reference)
"""C2Q (BiDAF-style) attention kernel for 8 TRN2 NeuronCores.

Pure data parallel: 64 batches sharded 8-per-core. Per batch b (reference):
    S = c @ c_w + (q @ q_w)^T + (c * cq_w) @ q^T + bias      (1024, 128)
    S1 = masked_softmax(S, q_mask, axis=j)
    S2 = masked_softmax(S1, c_mask, axis=i)
    A = S1 @ q ; Bm = S1 @ (S2^T @ c)
    out = [c | A | c*A | c*Bm]                                (1024, 512)

Key algebra: softmax over j is invariant to per-i constants, so the
c @ c_w term and the scalar bias CANCEL in S1 and never need computing.
Only R[j] = q @ q_w + log-mask(q_mask) survives (per-j), and it is a
per-partition bias in the transposed domain.

Device-side formulation (per batch):
    S^T[j,i] = qmodT.T @ cT   (bf16, 2 matmuls of N=512, qmodT stationary)
    E0T      = exp(S^T + R[j])              # ACT bias; stored bf16 [j, 1024]
    per chunk k: E0 = transpose(E0T chunk)  # PE; bf16 psum
                 E0_f32 -> SBUF with rowsum[i] via DVE accum_out
    rcprow = 1/rowsum ; G = exp(E0*rcprow + cmb[i])   # ACT scale+bias APs
    Traw[j,0:129] = sum_k G_k^T @ [c_k | 1]  (col 128 = colsum)
    Ts = Traw * (1/colsum)  -> bf16, next to q in the qq tile
    [Araw|Braw] = E0T_k.T @ [q | Ts]   (bf16, N=256)
    out chunk = [c | Araw*rcprow | c*Araw*rcprow | c*Braw*rcprow]
No max-subtraction needed: |S+R| <= ~30 so exp stays in range.
"""

import os
import numpy as np
import ml_dtypes

import concourse.bass as bass
import concourse.tile as tile
from concourse import bacc, mybir
from concourse.bass_utils import run_bass_kernel_spmd

F32 = mybir.dt.float32
BF16 = mybir.dt.bfloat16
AF = mybir.ActivationFunctionType
ALU = mybir.AluOpType

N_CORES = 8
B, CL, QL, D = 64, 1024, 128, 128
BPC = B // N_CORES          # batches per core
NK = CL // 128              # 128-row chunks per batch
MASK_NEG = -50.0            # exp(-50+eps) vanishes in f32 sums; in ACT range

LAST_RESULTS = None         # set by kernel() for test.py profiling


def _build_graph(loop_n=0):
    """loop_n=0: straight-line graph (production). loop_n=N>0: wrap the whole
    computation in a hardware For_i loop repeating it N times (timing only)."""
    nc = bacc.Bacc()

    c_ext = nc.declare_dram_parameter("c", [BPC, CL, D], F32, isOutput=False)
    cT_ext = nc.declare_dram_parameter("cT", [BPC, D, CL], BF16, isOutput=False)
    qq_ext = nc.declare_dram_parameter("qq", [BPC, D, 2 * QL], BF16, isOutput=False)
    cmR_ext = nc.declare_dram_parameter("cmR", [128, BPC * NK + BPC], F32, isOutput=False)
    id_ext = nc.declare_dram_parameter("ident", [128, 128], BF16, isOutput=False)
    out_ext = nc.declare_dram_parameter("out", [BPC, CL, 4 * D], F32, isOutput=True)

    with tile.TileContext(nc) as tc:
        with (
            tc.tile_pool(name="const", bufs=1) as const,
            tc.tile_pool(name="cbuf", bufs=3) as cbuf,
            tc.tile_pool(name="ctbuf", bufs=2) as ctbuf,
            tc.tile_pool(name="qq", bufs=2) as qqp,
            tc.tile_pool(name="e0tp", bufs=2) as e0tp,
            tc.tile_pool(name="e0p", bufs=12) as e0p,
            tc.tile_pool(name="gp", bufs=3) as gp,
            tc.tile_pool(name="stg", bufs=4) as stg,
            tc.tile_pool(name="rsp", bufs=2) as rsp,
            tc.tile_pool(name="stp", bufs=2, space=bass.MemorySpace.PSUM) as stp,
            tc.tile_pool(name="tpp", bufs=2, space=bass.MemorySpace.PSUM) as tpp,
            tc.tile_pool(name="trawp", bufs=1, space=bass.MemorySpace.PSUM) as trawp,
            tc.tile_pool(name="abp", bufs=3, space=bass.MemorySpace.PSUM) as abp,
        ):
            ident = const.tile([128, 128], BF16, tag="ident")
            nc.sync.dma_start(ident[:], id_ext[:])
            cmR = const.tile([128, BPC * NK + BPC], F32, tag="cmR")
            nc.sync.dma_start(cmR[:], cmR_ext[:])

            def _batch(b):
                # c tile: 8 groups of [128 ctx cols | ones col] -> [128, 8*129]
                c_t = cbuf.tile([128, NK * 129], F32, tag="c")
                cg = c_t[:].rearrange("p (k d) -> p k d", d=129)
                nc.vector.memset(cg[:, :, 128:129], 1.0)
                nc.sync.dma_start(
                    cg[:, :, 0:128],
                    c_ext[b].rearrange("(k p) d -> p k d", p=128),
                )
                cT_t = ctbuf.tile([128, CL], BF16, tag="cT")
                nc.sync.dma_start(cT_t[:], cT_ext[b])
                # qq tile: [qmodT | q | Ts]  (bf16)
                qq_t = qqp.tile([128, 3 * QL], BF16, tag="qq")
                nc.sync.dma_start(qq_t[:, 0:2 * QL], qq_ext[b])

                rowsum = rsp.tile([128, NK], F32, tag="rowsum")
                rcprow = rsp.tile([128, NK], F32, tag="rcprow")
                rcp2 = rsp.tile([128, 1], F32, tag="rcp2")

                # S^T = qmodT.T @ cT ; E0T = exp(S^T + R[j])  [j, 1024] bf16
                e0t_t = e0tp.tile([128, CL], BF16, tag="e0t")
                for h in range(2):
                    sp = stp.tile([128, 512], F32, tag="sp")
                    nc.tensor.matmul(
                        sp[:], qq_t[:, 0:QL], cT_t[:, h * 512:(h + 1) * 512]
                    )
                    nc.scalar.activation(
                        e0t_t[:, h * 512:(h + 1) * 512], sp[:], AF.Exp,
                        bias=cmR[:, BPC * NK + b:BPC * NK + b + 1],
                    )

                # per chunk: E0 natural (f32) + rowsum via DVE accum
                e0_l = []
                for k in range(NK):
                    ep = tpp.tile([128, 128], BF16, tag="ep")
                    nc.tensor.transpose(ep[:], e0t_t[:, k * 128:(k + 1) * 128], ident[:])
                    e0_t = e0p.tile([128, 128], F32, tag="e0")
                    nc.vector.tensor_scalar(
                        e0_t[:], ep[:], 1.0, 0.0, ALU.mult, ALU.add,
                        accum_out=rowsum[:, k:k + 1],
                    )
                    e0_l.append(e0_t)
                nc.vector.reciprocal(rcprow[:], rowsum[:])

                # G = exp(E0*rcprow + cmb) ; Traw accum (col 128 = colsum)
                traw = trawp.tile([128, 129], F32, tag="traw")
                for k in range(NK):
                    col = b * NK + k
                    g_t = gp.tile([128, QL], F32, tag="g")
                    nc.scalar.activation(
                        g_t[:], e0_l[k][:], AF.Exp,
                        bias=cmR[:, col:col + 1],
                        scale=rcprow[:, k:k + 1],
                    )
                    nc.tensor.matmul(
                        traw[:, 0:129], g_t[:], c_t[:, k * 129:(k + 1) * 129],
                        start=(k == 0), stop=(k == NK - 1),
                    )
                nc.vector.reciprocal(rcp2[:], traw[:, 128:129])
                nc.vector.tensor_scalar_mul(qq_t[:, 2 * QL:3 * QL], traw[:, 0:128], rcp2[:])

                # [Araw|Braw] = E0T_k.T @ [q | Ts] ; stage output chunk
                for k in range(NK):
                    ab = abp.tile([128, 2 * QL], F32, tag="ab")
                    nc.tensor.matmul(
                        ab[:], e0t_t[:, k * 128:(k + 1) * 128], qq_t[:, QL:3 * QL]
                    )
                    st = stg.tile([128, 4 * D], F32, tag="st")
                    cchunk = c_t[:, k * 129:k * 129 + 128]
                    # col 0:128 = c  (gpsimd)
                    nc.gpsimd.tensor_copy(st[:, 0:128], cchunk)
                    # col 128:256 = A = Araw * rcprow  (ACT copy-scale)
                    nc.scalar.activation(
                        st[:, 128:256], ab[:, 0:128], AF.Copy,
                        scale=rcprow[:, k:k + 1],
                    )
                    # col 256:384 = c*A  (DVE)
                    nc.vector.scalar_tensor_tensor(
                        st[:, 256:384], ab[:, 0:128], rcprow[:, k:k + 1],
                        cchunk, ALU.mult, ALU.mult,
                    )
                    # col 384:512 = c*Bm  (DVE)
                    nc.vector.scalar_tensor_tensor(
                        st[:, 384:512], ab[:, 128:256], rcprow[:, k:k + 1],
                        cchunk, ALU.mult, ALU.mult,
                    )
                    nc.sync.dma_start(out_ext[b, k * 128:(k + 1) * 128, :], st[:])

            if loop_n:
                with tc.For_i(0, loop_n, 1):
                    for b in range(BPC):
                        _batch(b)
            else:
                for b in range(BPC):
                    _batch(b)
    return nc


def _prep(c, q, c_mask, q_mask, c_weight, q_weight, cq_weight, bias):
    c = np.ascontiguousarray(np.asarray(c, dtype=np.float32))
    q = np.ascontiguousarray(np.asarray(q, dtype=np.float32))
    c_mask = np.asarray(c_mask)
    q_mask = np.asarray(q_mask)
    q_weight = np.asarray(q_weight, dtype=np.float32)
    cq_weight = np.asarray(cq_weight, dtype=np.float32)

    # host-side prep (tiny). NOTE: c@c_weight and bias cancel in softmax_j.
    s1 = (q.reshape(-1, D) @ q_weight).reshape(B, QL)          # (B, 128)
    R = s1 + np.where(q_mask > 0, 0.0, MASK_NEG).astype(np.float32)
    cmb = np.where(c_mask > 0, 0.0, MASK_NEG).astype(np.float32)  # (B, 1024)
    cT = np.ascontiguousarray(c.transpose(0, 2, 1)).astype(ml_dtypes.bfloat16)
    qmodT = np.ascontiguousarray(
        (q * cq_weight.reshape(1, 1, D)).transpose(0, 2, 1)
    ).astype(ml_dtypes.bfloat16)
    qT_rows = q.astype(ml_dtypes.bfloat16)                     # (B, 128, 128) [j, e]
    qq = np.concatenate([qmodT, qT_rows], axis=2)              # (B, 128, 256)

    in_maps = []
    for core in range(N_CORES):
        sl = slice(core * BPC, (core + 1) * BPC)
        cmT = cmb[sl].reshape(BPC, NK, 128).transpose(2, 0, 1).reshape(128, BPC * NK)
        cmR = np.ascontiguousarray(
            np.concatenate([cmT, R[sl].T], axis=1)             # (128, 64+8)
        )
        in_maps.append({
            "c": c[sl],
            "cT": cT[sl],
            "qq": np.ascontiguousarray(qq[sl]),
            "cmR": cmR,
            "ident": np.eye(128, dtype=ml_dtypes.bfloat16),
        })
    return in_maps


def make_in_maps():
    """For the local test/compare harness only (imports reference)."""
    import reference
    inputs = {k: np.asarray(v) for k, v in reference.setup_inputs().items()}
    return _prep(**inputs)


def kernel(c, q, c_mask, q_mask, c_weight, q_weight, cq_weight, bias):
    global LAST_RESULTS
    in_maps = _prep(c, q, c_mask, q_mask, c_weight, q_weight, cq_weight, bias)
    os.environ["BASS_NEVER_TRACE"] = "1"  # no NTFF hook in this container
    nc = _build_graph()
    nc.finalize()
    res = run_bass_kernel_spmd(nc, in_maps, core_ids=list(range(N_CORES)))
    LAST_RESULTS = (nc, in_maps)
    return np.concatenate([res.results[i]["out"] for i in range(N_CORES)], axis=0)

